# revision 1
# baseline (speedup 1.0000x reference)
"""Trainium2 Bass kernel for the siamese-kNN classification head.

Reference computation (B=256, N=2000, D=512, C=100):
    scores[b,n] = sigmoid(sum_d w_d * |a[b,d] - S[n,d]| + kb)
    out[b,c]    = (scores @ L)[b,c] / count_c     (0 where count_c == 0)

Strategy
--------
Data-parallel over the batch: core i handles rows 32*i .. 32*i+32 and needs
no collectives.  The pairwise |a-s| part (256*2000*512 elementwise ops) is
the dominant cost; it cannot be expressed as a matmul because of the abs.

We use the identity |x| = relu(2x) - x, which splits the score into a
nonlinear "slab" that the Vector engine (and, for a few rows, the Scalar
engine) produces in ONE instruction per element-touch, plus a separable
linear part that folds into a tiny rank-2 correction matmul:

    w_d*|a-S| = sign(w_d)*relu(A''-S'') - w_d*a + w_d*S
    with A'' = 2|w| (.) a,  S'' = 2|w| (.) S   (pre-scaled on host, bf16).

Layout: d on partitions (4 chunks of 128), n on the free dim.
  - DVE rows:  tensor_scalar(S''chunk, sub A''[:,b], min 0) = -relu(A''-S'')
               bf16 in/out, ~730ns per [128,2000] slab.
  - ACT rows:  activation(Relu, scale=-1, bias=A''[:,b]) = +relu(A''-S'')
  - PE reduces each slab over d into PSUM rows [32, seg].  The stationary is
    a [128, 32] window of a [128, 63] bf16 tile that is zero everywhere
    except column 31 = -/+ sign; window 31-b places the sign at column b, so
    row b accumulates the signed reduction and every other row accumulates
    an exact 0.  A rank-2 f32r correction matmul pre-loads
    kb - w.a_b + (w.S)_n (start=True); chunk-major loop order so compute
    starts as soon as the first S'' chunk lands.
Then sigmoid straight out of PSUM (ACT, bf16), 16 PE-transposes of the
[32,2000] score tile packed into ONE PSUM bank, one PSUM->SBUF copy, a
final bf16 [n,32]^T @ labels[n,100] matmul, and a scale by the per-class
1/count (host-prepared with divide-no-nan semantics, like the other O(N*D)
support-data prep: w-scaling, S@w, a@w).

PE is the structural bottleneck: it must ingest every slab element at 128
elem/cycle -> ~110us/core minimum for the 33.5M per-core slab elements.
"""

import sys

for _p in ("/opt/trn_rl_repo", "/root/.axon_site/_ro/trn_rl_repo"):
    if _p not in sys.path:
        sys.path.append(_p)

import numpy as np

B, N, D, C = 256, 2000, 512, 100
NP = 2048                  # label rows padded to 16 full chunks
NCORES = 8
BSH = B // NCORES          # 32 batch rows per core
DCH = D // 128             # 4 d-chunks
NSEG = 4                   # PSUM free-dim segments
SEG = N // NSEG            # 500
NLAB = NP // 128           # 16 label chunks
N_DVE = 28                 # rows produced on DVE (rest on ACT)

_CACHE = {}


def _use_dve(b):
    return ((b + 1) * N_DVE) // BSH != (b * N_DVE) // BSH


def _split_multi_waits(nc):
    """TRN2 TPB instructions encode at most ONE semaphore wait, but Tile can
    attach several (e.g. the tail drain, or an op whose inputs arrived on two
    DMA queues); this walrus build refuses those.  Splitting the extras into
    single-wait NOPs directly before the instruction on the same engine is
    semantically identical (engines execute their block instructions in
    order)."""
    from concourse import mybir

    for fn in nc.m.functions:
        for bb in fn.blocks:
            out = []
            for inst in bb.instructions:
                si = inst.sync_info
                if si is not None and si.on_wait and len(si.on_wait) > 1:
                    waits = list(si.on_wait)
                    for j, w in enumerate(waits[:-1]):
                        out.append(mybir.InstNoOp(
                            name=f"{inst.name}-sw{j}", engine=inst.engine,
                            sync_info=mybir.SyncInfo(on_wait=[w], on_update=[]),
                            ins=[], outs=[]))
                    inst.sync_info = mybir.SyncInfo(
                        on_wait=[waits[-1]], on_update=list(si.on_update))
                out.append(inst)
            bb.instructions = out


def _build_nc():
    import concourse.bass as bass
    import concourse.tile as tile
    from concourse import mybir

    f32 = mybir.dt.float32
    f32r = mybir.dt.float32r
    bf16 = mybir.dt.bfloat16
    nc = bass.Bass()

    s2t_d = nc.declare_dram_parameter("s2t", [D, N], bf16, isOutput=False)
    a2t_d = nc.declare_dram_parameter("a2t", [DCH, 128, BSH], f32, isOutput=False)
    # sliding-window sign tiles: zero except col 31 = -sign / +sign per chunk
    sgnn_d = nc.declare_dram_parameter("sgnn", [128, DCH, 63], bf16, isOutput=False)
    sgnp_d = nc.declare_dram_parameter("sgnp", [128, DCH, 63], bf16, isOutput=False)
    clhs_d = nc.declare_dram_parameter("clhs", [2, BSH], f32r, isOutput=False)
    crhs_d = nc.declare_dram_parameter("crhs", [2, N], f32r, isOutput=False)
    # labels packed [128, chunk, C] (host-padded to 2048 rows), bf16 (0/1 exact)
    lab_d = nc.declare_dram_parameter("labels", [128, NLAB, C], bf16, isOutput=False)
    ident_d = nc.declare_dram_parameter("ident", [32, 32], bf16, isOutput=False)
    recb_d = nc.declare_dram_parameter("recb", [BSH, C], f32, isOutput=False)
    out_d = nc.declare_dram_parameter("out", [BSH, C], f32, isOutput=True)

    with tile.TileContext(nc) as tc:
        with (
            tc.tile_pool(name="const", bufs=1) as const,
            tc.tile_pool(name="dslab", bufs=4) as dpool,
            tc.tile_pool(name="aslab", bufs=3) as apool,
            tc.tile_pool(name="bank", bufs=8, space="PSUM") as bankp,
        ):
            # ---- constant loads, in priority order (each dma_start costs
            # ~650ns of serial issue time on the Sync sequencer)
            s2t0 = const.tile([128, N], bf16, name="s2t0", tag="s2t0")
            nc.sync.dma_start(s2t0[:], s2t_d[0:128, :])
            a2t = const.tile([128, DCH * BSH], f32, name="a2t", tag="a2t")
            nc.sync.dma_start(
                a2t[:].rearrange("p (c b) -> p c b", c=DCH),
                a2t_d[:].rearrange("c p b -> p c b"),
            )
            clhs = const.tile([2, BSH], f32r, name="clhs", tag="clhs")
            nc.sync.dma_start(clhs[:], clhs_d[:])
            crhs = const.tile([2, N], f32r, name="crhs", tag="crhs")
            nc.sync.dma_start(crhs[:], crhs_d[:])
            sgnn = const.tile([128, DCH, 63], bf16, name="sgnn", tag="sgnn")
            nc.sync.dma_start(sgnn[:], sgnn_d[:])
            sgnp = const.tile([128, DCH, 63], bf16, name="sgnp", tag="sgnp")
            nc.sync.dma_start(sgnp[:], sgnp_d[:])
            s2t123 = const.tile([128, 3 * N], bf16, name="s2t123", tag="s2t123")
            nc.sync.dma_start(
                s2t123[:].rearrange("p (c n) -> p c n", c=3),
                s2t_d[:].rearrange("(c p) n -> p c n", p=128)[:, 1:4, :],
            )
            ident = const.tile([32, 32], bf16, name="ident", tag="ident")
            nc.sync.dma_start(ident[:], ident_d[:])
            labs = const.tile([128, NLAB, C], bf16, name="labs", tag="labs")
            nc.sync.dma_start(labs[:], lab_d[:])
            recb = const.tile([BSH, C], f32, name="recb", tag="recb")
            nc.sync.dma_start(recb[:], recb_d[:])
            s2t = [s2t0] + [
                s2t123[:, ch * N : (ch + 1) * N] for ch in range(3)
            ]

            # ---- score accumulation (chunk-major: start after chunk 0 lands)
            psc = [
                bankp.tile([BSH, SEG], f32, name=f"psc{s}", tag="bank")
                for s in range(NSEG)
            ]
            for s in range(NSEG):
                nc.tensor.matmul(
                    psc[s][:], clhs[:], crhs[:, SEG * s : SEG * (s + 1)],
                    start=True, stop=False,
                )

            for ch in range(DCH):
                for b in range(BSH):
                    if _use_dve(b):
                        slab = dpool.tile([128, N], bf16, name="dslab", tag="dslab")
                        nc.vector.tensor_scalar(
                            slab[:], s2t[ch][:],
                            a2t[:, ch * BSH + b : ch * BSH + b + 1], 0.0,
                            mybir.AluOpType.subtract, mybir.AluOpType.min,
                        )
                        sg = sgnn
                    else:
                        slab = apool.tile([128, N], bf16, name="aslab", tag="aslab")
                        nc.scalar.activation(
                            slab[:], s2t[ch][:],
                            mybir.ActivationFunctionType.Relu,
                            bias=a2t[:, ch * BSH + b : ch * BSH + b + 1],
                            scale=-1.0,
                        )
                        sg = sgnp
                    lhs = sg[:, ch, 31 - b : 63 - b]
                    for s in range(NSEG):
                        nc.tensor.matmul(
                            psc[s][:],
                            lhs,
                            slab[:, SEG * s : SEG * (s + 1)],
                            start=False, stop=(ch == DCH - 1),
                            skip_group_check=True,
                        )

            # ---- sigmoid (PSUM -> SBUF, bf16) ----
            ssig = const.tile([BSH, N], bf16, name="ssig", tag="ssig")
            for s in range(NSEG):
                nc.scalar.activation(
                    ssig[:, SEG * s : SEG * (s + 1)], psc[s][:],
                    mybir.ActivationFunctionType.Sigmoid,
                )

            # ---- 16 transposes into ONE PSUM bank, one copy, final matmuls
            tpall = bankp.tile([128, NLAB * BSH], bf16, name="tpall", tag="bank")
            for k in range(NLAB):
                pk = min(128, N - 128 * k)
                nc.tensor.transpose(
                    tpall[:pk, BSH * k : BSH * k + BSH],
                    ssig[:, 128 * k : 128 * k + pk], ident[:],
                )
            sct = const.tile([128, NLAB * BSH], bf16, name="sct", tag="sct")
            nc.vector.tensor_copy(sct[:], tpall[:])
            out_ps = bankp.tile([BSH, C], f32, name="out_ps", tag="bank")
            for k in range(NLAB):
                pk = min(128, N - 128 * k)
                nc.tensor.matmul(
                    out_ps[:], sct[:pk, BSH * k : BSH * k + BSH],
                    labs[:pk, k, :],
                    start=(k == 0), stop=(k == NLAB - 1),
                )

            # ---- divide by counts, write out ----
            out_s = const.tile([BSH, C], f32, name="out_s", tag="out_s")
            nc.vector.tensor_mul(out_s[:], out_ps[:], recb[:])
            nc.sync.dma_start(out_d[:], out_s[:])

    _split_multi_waits(nc)
    return nc


def _prep_host(inputs, support_tensors, support_labels, kernel_w, kernel_b):
    import ml_dtypes

    bf16 = ml_dtypes.bfloat16
    a = np.asarray(inputs, dtype=np.float32)
    S = np.asarray(support_tensors, dtype=np.float32)
    L = np.asarray(support_labels, dtype=np.float32)
    w = np.asarray(kernel_w, dtype=np.float32)
    kb = np.float32(np.asarray(kernel_b, dtype=np.float32))

    aw = 2.0 * np.abs(w)
    sgn = np.sign(w).astype(np.float32)
    s2t = np.ascontiguousarray((S * aw[None, :]).T).astype(bf16)   # [D, N]
    wS = (S @ w).astype(np.float32)                                # [N]
    wa = (a @ w).astype(np.float32)                                # [B]
    a2 = a * aw[None, :]                                           # [B, D]

    # sliding-window sign tiles [128, DCH, 63]: col 31 = -/+ sign chunk
    sgn_chunks = sgn.reshape(DCH, 128).T                           # [128, DCH]
    sgnn = np.zeros((128, DCH, 63), dtype=np.float32)
    sgnn[:, :, 31] = -sgn_chunks
    sgnp = np.zeros((128, DCH, 63), dtype=np.float32)
    sgnp[:, :, 31] = sgn_chunks
    crhs = np.empty((2, N), dtype=np.float32)
    crhs[0] = 1.0
    crhs[1] = wS
    labp = np.zeros((NP, C), dtype=np.float32)
    labp[:N] = L
    labp = np.ascontiguousarray(
        labp.reshape(NLAB, 128, C).transpose(1, 0, 2)).astype(bf16)
    ident = np.eye(32, dtype=bf16)
    counts = L.sum(axis=0)
    recip = np.where(counts != 0, 1.0 / np.maximum(counts, 1e-30), 0.0)
    recb = np.broadcast_to(recip.astype(np.float32), (BSH, C)).copy()

    shared = {
        "s2t": s2t, "sgnn": sgnn.astype(bf16), "sgnp": sgnp.astype(bf16),
        "crhs": crhs, "labels": labp, "ident": ident, "recb": recb,
    }
    in_maps = []
    for c in range(NCORES):
        rows = slice(BSH * c, BSH * (c + 1))
        a2t_c = np.ascontiguousarray(
            a2[rows].T.reshape(DCH, 128, BSH))                     # [DCH,128,BSH]
        clhs_c = np.empty((2, BSH), dtype=np.float32)
        clhs_c[0] = kb - wa[rows]
        clhs_c[1] = 1.0
        in_maps.append(dict(shared, a2t=a2t_c, clhs=clhs_c))
    return in_maps


def kernel(**inputs) -> np.ndarray:
    from concourse.bass_utils import run_bass_kernel_spmd

    if "nc" not in _CACHE:
        _CACHE["nc"] = _build_nc()
    nc = _CACHE["nc"]

    in_maps = _prep_host(
        inputs["inputs"], inputs["support_tensors"], inputs["support_labels"],
        inputs["kernel_w"], inputs["kernel_b"],
    )
    res = run_bass_kernel_spmd(nc, in_maps, list(range(NCORES)))
    return np.concatenate([res.results[i]["out"] for i in range(NCORES)], axis=0)

